# revision 10
# baseline (speedup 1.0000x reference)
"""Trainium2 Bass kernel for nn_CMultiHeadAttention_59614146068936.

Sharding: 8 cores = (batch b, i-half). Each core computes, for its
(b, 1024 rows of i), the full [8, 1024, 2048] probs shard and the
[1024, 512] out shard.

Math per core (rows m = h*16+i_sub, h-major; groups of 16 i):
  A[(i,c,tb), j]  = |dk_{top/bot}[j,c] - dq[i,c]|           (DVE ts 2x)
  M'[(i,c,tb), j] = A * score[j,tb,c]                       (DVE tt)
  dwpre/dbpre[(h,i), j] = blockdiag(Ww/Wb).T @ M'           (PE)
  dw  = ln(exp(dwpre + bw) + 1)          [softplus]         (ACT x2)
  s   = (dw * ka[h,j]) * qa[h,i] + dbpre                    (DVE tt+stt)
  p   = exp(s), denom = row-sum (accum_out), probs = p/denom(ACT+DVE)
  pT  = PE-transpose(probs) -> PSUM -> SBUF                 (PE + copy)
  att = sum_j probs.T-blocks @ vh-blocks                    (PE)
  out = att @ Wo + bo                                       (PE)

Projections qa/ka/vh and all layout replication are host-side numpy
(pure data layout + tiny matmuls; the O(L^2) work is all on device).
"""
import os
import sys
import numpy as np

sys.path.insert(0, "/opt/trn_rl_repo")

B, L, D, H, C = 4, 2048, 512, 8, 3
DH = D // H
GI = 16               # i's per group
KP = 6                # (c, tb) rows per i
ROWS_A = GI * KP      # 96
IB = 4                # groups per i-batch (64 i rows)
F32 = np.float32


# ---------------------------------------------------------------- host prep

def _host_proj(inputs):
    q = np.asarray(inputs['q'], F32)
    k = np.asarray(inputs['k'], F32)
    v = np.asarray(inputs['v'], F32)
    qa = (q @ np.asarray(inputs['Wq'], F32) + np.asarray(inputs['bq'], F32))
    ka = (k @ np.asarray(inputs['Wk'], F32) + np.asarray(inputs['bk'], F32))
    vh = (v @ np.asarray(inputs['Wv'], F32) + np.asarray(inputs['bv'], F32))
    return qa.transpose(0, 2, 1).copy(), ka.transpose(0, 2, 1).copy(), vh


def _prep_core(inputs, qa, ka, vh, b, i0, Li, Lj):
    """Build the in_map (numpy arrays) for one core."""
    NG = Li // GI
    d_q = np.asarray(inputs['d_q'], F32)
    d_kt = np.asarray(inputs['d_k_top'], F32)
    d_kb = np.asarray(inputs['d_k_bot'], F32)
    d_ks = np.asarray(inputs['d_k_score'], F32)
    Ww = np.asarray(inputs['Ww'], F32)
    Wb = np.asarray(inputs['Wb'], F32)
    bw = np.asarray(inputs['bw'], F32)
    bo = np.asarray(inputs['bo'], F32)
    Wo = np.asarray(inputs['Wo'], F32)

    p = np.arange(ROWS_A)
    isub_p, k_p = p // KP, p % KP
    c_p, tb_p = k_p // 2, k_p % 2
    m = np.arange(128)
    h_m, i_m = m // 16, m % 16

    dkTB = np.where(tb_p[:, None] == 0,
                    d_kt[b][:Lj, :].T[c_p, :],
                    d_kb[b][:Lj, :].T[c_p, :]).astype(F32)         # [96, Lj]
    sTB = d_ks[b][:Lj][:, tb_p, c_p].T.astype(F32).copy()          # [96, Lj]
    ii = i0 + (np.arange(NG) * GI)[None, :] + isub_p[:, None]
    dq_cols = d_q[b][ii, c_p[:, None]].astype(F32).copy()          # [96, NG]
    ndq_cols = (-dq_cols).copy()                                   # [96, NG]
    iiq = i0 + (np.arange(NG) * GI)[None, :] + i_m[:, None]
    qa_cols = qa[b][h_m[:, None], iiq].astype(F32).copy()          # [128, NG]
    ka_rep = ka[b][h_m, :Lj].astype(F32).copy()                    # [128, Lj]
    njc = Lj // 128
    vh_jc = vh[b][:Lj].reshape(njc, 128, D).transpose(1, 0, 2) \
        .reshape(128, njc * D).copy()                              # [128, njc*512]
    blk = (isub_p[:, None] == i_m[None, :])
    W6w = (blk * Ww[c_p][:, h_m]).astype(F32).copy()               # [96, 128]
    W6b = (blk * Wb[c_p][:, h_m]).astype(F32).copy()
    bw_col = bw[h_m].reshape(128, 1).astype(F32).copy()
    bo_rep = np.broadcast_to(bo, (64, D)).astype(F32).copy()
    Wo_ch = Wo.reshape(4, 128, D).transpose(1, 0, 2).reshape(128, 4 * D).copy()
    ident = np.eye(128, dtype=F32)
    return dict(dkTB=dkTB, sTB=sTB, ndq_cols=ndq_cols, qa_cols=qa_cols,
                ka_rep=ka_rep, vh_jc=vh_jc, W6w=W6w, W6b=W6b,
                bw_col=bw_col, bo_rep=bo_rep, Wo_ch=Wo_ch, ident=ident)


# ---------------------------------------------------------------- bass build

def _patch_act_tables():
    """Make every ACT func we use resolve to natural_log_exp_and_others so
    bacc emits exactly one ACT_TABLE_LOAD (default mapping ping-pongs
    exp_and_others <-> natural_log at ~2.7us per switch)."""
    import concourse.bacc as bacc
    import concourse.hw_specs as hw_specs
    if getattr(bacc, "_act_tables_patched", False):
        return
    orig = hw_specs.get_activation_tables

    def patched(arch):
        tables = dict(orig(arch))
        import concourse.mybir as mybir
        A = mybir.ActivationFunctionType
        mine = {A.Exp, A.Ln, A.Abs, A.Copy, A.Identity}
        out = {}
        for name, fns in tables.items():
            if name == "natural_log_exp_and_others":
                out[name] = set(fns)
            else:
                out[name] = set(fns) - mine
        return out

    bacc.get_activation_tables = patched
    bacc._act_tables_patched = True


def build_nc(Li, Lj, num_devices=8, enable_asserts=False):
    """Build + bacc-compile the SPMD program for one core shape."""
    from contextlib import ExitStack
    import concourse.bass as bass
    import concourse.bacc as bacc
    import concourse.tile as tile
    import concourse.mybir as mybir
    _patch_act_tables()

    dt = mybir.dt.float32
    Alu = mybir.AluOpType
    Act = mybir.ActivationFunctionType
    NG = Li // GI
    NQ = Lj // 512          # j-quarters
    NJC = Lj // 128         # j-chunks of 128
    NB = Li // (IB * GI)    # i-batches

    nc = bacc.Bacc("TRN2", target_bir_lowering=False, debug=False,
                   enable_asserts=enable_asserts, num_devices=num_devices)

    def din(name, shape):
        return nc.dram_tensor(name, shape, dt, kind="ExternalInput").ap()

    t_dkTB = din("dkTB", [ROWS_A, Lj])
    t_sTB = din("sTB", [ROWS_A, Lj])
    t_dq = din("ndq_cols", [ROWS_A, NG])
    t_qa = din("qa_cols", [128, NG])
    t_ka = din("ka_rep", [128, Lj])
    t_vh = din("vh_jc", [128, NJC * D])
    t_W6w = din("W6w", [ROWS_A, 128])
    t_W6b = din("W6b", [ROWS_A, 128])
    t_bw = din("bw_col", [128, 1])
    t_bo = din("bo_rep", [64, D])
    t_Wo = din("Wo_ch", [128, 4 * D])
    t_id = din("ident", [128, 128])
    t_probs = nc.dram_tensor("probs", [H, Li, Lj], dt, kind="ExternalOutput").ap()
    t_out = nc.dram_tensor("out", [Li, D], dt, kind="ExternalOutput").ap()

    with tile.TileContext(nc) as tc, ExitStack() as ctx:
        const = ctx.enter_context(tc.tile_pool(name="const", bufs=1))
        am = ctx.enter_context(tc.tile_pool(name="am", bufs=2))
        small = ctx.enter_context(tc.tile_pool(name="small", bufs=2))
        big = ctx.enter_context(tc.tile_pool(name="big", bufs=2))
        pts = ctx.enter_context(tc.tile_pool(name="pts", bufs=1))
        psmm = ctx.enter_context(tc.tile_pool(name="psmm", bufs=4, space="PSUM"))
        pst1 = ctx.enter_context(tc.tile_pool(name="pst1", bufs=2, space="PSUM"))
        psatt = ctx.enter_context(tc.tile_pool(name="psatt", bufs=1, space="PSUM"))
        pswo = ctx.enter_context(tc.tile_pool(name="pswo", bufs=1, space="PSUM"))

        def cload(ap, shape, name):
            tl = const.tile(shape, dt, tag=name)
            nc.sync.dma_start(out=tl[:], in_=ap)
            return tl

        sb_dkTB = cload(t_dkTB, [ROWS_A, Lj], "dkTB")
        sb_sTB = cload(t_sTB, [ROWS_A, Lj], "sTB")
        sb_dq = cload(t_dq, [ROWS_A, NG], "dq")
        sb_qa = cload(t_qa, [128, NG], "qa")
        sb_ka = cload(t_ka, [128, Lj], "ka")
        sb_vh = cload(t_vh, [128, NJC * D], "vh")
        sb_W6w = cload(t_W6w, [ROWS_A, 128], "W6w")
        sb_W6b = cload(t_W6b, [ROWS_A, 128], "W6b")
        sb_bw = cload(t_bw, [128, 1], "bw")
        sb_bo = cload(t_bo, [64, D], "bo")
        sb_Wo = cload(t_Wo, [128, 4 * D], "Wo")
        sb_id = cload(t_id, [128, 128], "id")

        pT = None
        for g in range(NG):
            # ---- A = |dk - dq| (ACT Abs, bias = -dq), M' = A*s (Pool) --
            A = am.tile([ROWS_A, Lj], dt, tag="A")
            nc.scalar.activation(A[:], sb_dkTB[:], Act.Abs,
                                 bias=t_sl(sb_dq, g), scale=1.0)
            Mp = am.tile([ROWS_A, Lj], dt, tag="M")
            nc.gpsimd.tensor_mul(Mp[:], A[:], sb_sTB[:])

            p_un = big.tile([128, Lj], dt, tag="p_un")
            sums = small.tile([128, NQ], dt, tag="sums")

            for q in range(NQ):
                js = slice(q * 512, (q + 1) * 512)
                # ---- dwpre / dbpre -------------------------------------
                dw_ps = psmm.tile([128, 512], dt, tag="mm")
                db_ps = psmm.tile([128, 512], dt, tag="mm")
                nc.tensor.matmul(dw_ps[:], sb_W6w[:], Mp[:, js],
                                 start=True, stop=True)
                nc.tensor.matmul(db_ps[:], sb_W6b[:], Mp[:, js],
                                 start=True, stop=True)
                # ---- softplus: dw = ln(exp(dwpre + bw) + 1) ------------
                e1 = small.tile([128, 512], dt, tag="e1")
                nc.scalar.activation(e1[:], dw_ps[:], Act.Exp,
                                     bias=sb_bw[:, 0:1], scale=1.0)
                dw = small.tile([128, 512], dt, tag="dw")
                nc.scalar.activation(dw[:], e1[:], Act.Ln, bias=1.0, scale=1.0)
                # ---- scores = (dw*ka)*qa + db --------------------------
                u = small.tile([128, 512], dt, tag="u")
                if (g * NQ + q) % 2 == 0:
                    nc.vector.tensor_mul(u[:], dw[:], sb_ka[:, js])
                else:
                    nc.gpsimd.tensor_mul(u[:], dw[:], sb_ka[:, js])
                sc = small.tile([128, 512], dt, tag="sc")
                nc.vector.scalar_tensor_tensor(out=sc[:], in0=u[:],
                                               scalar=t_sl(sb_qa, g),
                                               in1=db_ps[:],
                                               op0=Alu.mult, op1=Alu.add)
                # ---- p_un = exp(scores), partial row sums --------------
                nc.scalar.activation(p_un[:, js], sc[:], Act.Exp,
                                     accum_out=sums[:, q:q + 1])

            # ---- softmax normalize ------------------------------------
            den = small.tile([128, 1], dt, tag="den")
            nc.vector.tensor_reduce(den[:], sums[:], mybir.AxisListType.X,
                                    Alu.add)
            rec = small.tile([128, 1], dt, tag="rec")
            nc.vector.reciprocal(rec[:], den[:])
            probs = big.tile([128, Lj], dt, tag="probs")
            nc.vector.tensor_scalar(out=probs[:], in0=p_un[:],
                                    scalar1=rec[:, 0:1], scalar2=None,
                                    op0=Alu.mult)
            # ---- DMA probs shard --------------------------------------
            nc.sync.dma_start(out=t_probs[:, g * GI:(g + 1) * GI, :],
                              in_=probs[:])

            # ---- T1: probs -> pT (transposed, per j-chunk) ------------
            if g % IB == 0:
                pT = pts.tile([128, NJC * 512], dt, tag="pT")
            gg = g % IB
            for q4 in range((NJC + 3) // 4):
                nt = min(4, NJC - q4 * 4)
                t1 = pst1.tile([128, 512], dt, tag="t1")
                for jj in range(nt):
                    jc = q4 * 4 + jj
                    nc.tensor.transpose(t1[:, jj * 128:(jj + 1) * 128],
                                        probs[:, jc * 128:(jc + 1) * 128],
                                        sb_id[:])
                # scatter-copy PSUM -> pT[jc-block, h-major cols]
                src = t1.rearrange("p (jj h i) -> p jj h i", jj=nt, h=H) \
                    if nt > 1 else t1[:, 0:128].rearrange(
                        "p (h i) -> p h i", h=H)
                dst = pT.rearrange("p (jc h i) -> p jc h i", jc=NJC, h=H)
                if nt > 1:
                    dstv = dst[:, q4 * 4:q4 * 4 + nt, :,
                               gg * GI:(gg + 1) * GI]
                else:
                    dstv = dst[:, q4 * 4, :, gg * GI:(gg + 1) * GI]
                nc.vector.tensor_copy(dstv, src)

            # ---- per i-batch: PV, att @ Wo + bo, out ------------------
            if g % IB == IB - 1:
                ib = g // IB
                att_ps = psatt.tile([64, 512], dt, tag="att")
                for h in range(H):
                    for jc in range(NJC):
                        lhsT = pT[:, jc * 512 + h * 64:jc * 512 + h * 64 + 64]
                        rhs = sb_vh[:, jc * D + h * DH:jc * D + (h + 1) * DH]
                        nc.tensor.matmul(att_ps[:, h * 64:(h + 1) * 64],
                                         lhsT, rhs,
                                         start=(jc == 0),
                                         stop=(jc == NJC - 1))
                att_sb = small.tile([64, 512], dt, tag="att_sb")
                nc.scalar.copy(att_sb[:], att_ps[:])
                atT_ps = pst1.tile([128, 256], dt, tag="t1")
                for ch in range(4):
                    nc.tensor.transpose(atT_ps[:, ch * 64:(ch + 1) * 64],
                                        att_sb[:, ch * 128:(ch + 1) * 128],
                                        sb_id[0:64, 0:64])
                atT_sb = small.tile([128, 256], dt, tag="atT_sb")
                nc.scalar.copy(atT_sb[:], atT_ps[:])
                out_ps = pswo.tile([64, 512], dt, tag="wo")
                for ch in range(4):
                    nc.tensor.matmul(out_ps[:],
                                     atT_sb[:, ch * 64:(ch + 1) * 64],
                                     sb_Wo[:, ch * D:(ch + 1) * D],
                                     start=(ch == 0), stop=(ch == 3))
                out_sb = small.tile([64, 512], dt, tag="out_sb")
                nc.vector.scalar_tensor_tensor(out=out_sb[:], in0=out_ps[:],
                                               scalar=1.0, in1=sb_bo[:],
                                               op0=Alu.mult, op1=Alu.add)
                nc.sync.dma_start(
                    out=t_out[ib * 64:(ib + 1) * 64, :], in_=out_sb[:])

    nc.compile()
    return nc


def t_sl(tile_, g):
    return tile_[:, g:g + 1]


# ---------------------------------------------------------------- entry

_NC_CACHE = {}


def kernel(**inputs):
    from concourse import bass_utils

    Li, Lj = L // 2, L
    qa, ka, vh = _host_proj(inputs)
    in_maps = []
    for core in range(8):
        b, ih = core // 2, core % 2
        in_maps.append(_prep_core(inputs, qa, ka, vh, b, ih * Li, Li, Lj))

    key = (Li, Lj)
    if key not in _NC_CACHE:
        _NC_CACHE[key] = build_nc(Li, Lj)
    nc = _NC_CACHE[key]

    res = bass_utils.run_bass_kernel_spmd(nc, in_maps, list(range(8)))

    out = np.zeros((B, L, D), F32)
    probs = np.zeros((B, H, L, L), F32)
    for core in range(8):
        b, ih = core // 2, core % 2
        r = res.results[core]
        probs[b, :, ih * Li:(ih + 1) * Li, :] = r["probs"]
        out[b, ih * Li:(ih + 1) * Li, :] = r["out"]
    return out, probs
